# revision 2
# baseline (speedup 1.0000x reference)
"""Trainium2 Bass kernel for nn_AttentionModel (4-layer dense transformer).

Contract: kernel(**inputs) takes FULL unsharded inputs (as produced by
setup_inputs) and returns the FULL output [N, L, V] fp32.

Sharding: data-parallel over batch N=8 across the 8 NeuronCores — each core
runs the complete transformer for one batch element (identical NEFF, per-core
tokens). No collectives needed; the host stacks the per-core outputs.

Per-core dataflow (L=1024, F=512, H=8, KD=QD=64, NL=4, V=1024):
  - embedding: indirect-DMA gather of embed rows by token -> x0 natural [L, F]
  - activations kept in two layouts:
      natural [l(128-part) x F]  - for layernorm / residual / softmax scales
      T       [F(128-part) x L]  - as matmul operands (contraction on partitions)
    PE-transposes (with identity) convert between them.
  - per layer:
      kT = Wk^T x^T, vT = Wv^T x^T (T layout, f32r matmuls)
      q  = x Wq (natural layout) stored as [j-chunk, head, 65] with a ones
           column so the attend matmul also produces softmax row-sums
      scores^T[j,i] = v k^T per head (K=64 f32r matmuls on disjoint PE
           row-groups per head pair, causal tiles only; diagonal-group tiles
           compute only the needed column range)
      att_u = exp(scores^T - 5) in fp16 (ACT, psum->sbuf; the -5 keeps exp in
           fp16 range and cancels in the softmax ratio); diagonal tiles
           triangle-zeroed in place with gpsimd affine_select (keep j<=i)
      x_new[i-block, head pair] = att_u^T @ [q | 1] (fp16 matmuls, one psum
           bank per pair): col 64 of each head = softmax row-sum; one strided
           reciprocal + one 0-stride-broadcast multiply normalizes during the
           psum->sbuf copy
      x_newT via PE transposes; MLP h1T = relu(W1^T x_newT + b1) (ACT bias);
      h = h1T^T W2; y = LN(x + h) (bn_stats/bn_aggr per chunk,
      rstd = exp(-0.5 ln(var+eps)); per-chunk vs batched is selectable via
      LN_BATCH and measured equal); yT via PE transposes
  - unembed: logits = x4 Wout + bout, DMA'd out per [128, 512] tile.

Matmul dtypes: projections/scores in float32r (1 cyc/col at N>=256, ~1.6e-4
scale-rel per matmul, matching the device fp32 matmul envelope); attention
attend in fp16 (att in [0,1] after the -5 shift, q well within fp16 range).
The ACT table-set choice is pinned (see _Bacc) so Exp/Ln/Relu/Copy share one
loaded set - no per-layer ~2.7us table swaps.
"""

import numpy as np

import concourse.bass as bass
import concourse.mybir as mybir
import concourse.tile as tile
from concourse import bacc
from concourse.bass_utils import run_bass_kernel_spmd
from concourse.masks import make_identity, make_upper_triangular

# Model dims (hardcoded per the problem spec)
V, F, NL, H, KD, QD = 1024, 512, 4, 8, 64, 64
N, L = 8, 1024
HQ = H * QD  # 512
P = 128
FC = F // P      # 4 f-chunks
LB = L // P      # 8 l-blocks of 128
NCORES = 8

f32 = mybir.dt.float32
f32r = mybir.dt.float32r
f16 = mybir.dt.float16
i32 = mybir.dt.int32
AF = mybir.ActivationFunctionType
OP = mybir.AluOpType

_NC_CACHE: dict = {}
ABLATE = "none"  # perf-analysis knob: none|scores|attend|transposes
LN_BATCH = False  # batch the LN ln/exp across the 8 l-chunks
TR_SPLIT = False  # alternate transpose psum->sbuf copies between DVE and ACT
EXPP_BUFS = 36  # in-flight fp16 att tiles (score->exp->attend pipeline
# depth); 36 beat 28 by ~6% in same-process A/B - lets the next head
# pair's scores+exps run fully ahead of the current pair's attend
PSUM_CFG = (3, 2, 3)  # bufs for (pp, pa, pt) pools, <= 8 banks total;
# (3,2,3) beat (4,2,2) by ~2.5% in same-process A/B: the PE-transpose ->
# DVE-copy pipeline wants 3 banks more than the score pipeline wants 4


class _Bacc(bacc.Bacc):
    """Bacc with activation-table-set selection pinned to
    natural_log_exp_and_others (contains Exp, Ln, Relu, Copy — everything this
    kernel uses) so the load-insertion pass emits one table load instead of
    thrashing between per-function sets (~2.7us per swap)."""

    _ACT_FUNCS = None  # set lazily to avoid import order issues

    def insert_act_table_loads(self):
        from concourse.hw_specs import get_activation_tables
        import concourse.mybir as _mb

        has_activation = any(
            isinstance(i, _mb.InstActivation)
            for b in self.main_func.blocks
            for i in b.instructions
        )
        if not has_activation:
            return
        keep = {AF.Exp, AF.Ln, AF.Relu, AF.Copy}
        chosen = "natural_log_exp_and_others"
        full = get_activation_tables(self.m.arch)
        assert keep <= full[chosen], (chosen, keep - full[chosen])
        tables = [
            (name, (fns if name == chosen else fns - keep))
            for name, fns in full.items()
        ]
        import bass_rust as _bass_rust
        _bass_rust.insert_act_table_loads(self, tables)


def _ln_apply(nc, y, b, mv8, rstd8, use_gamma, use_beta, gamma_b, beta_b):
    t = y[:, b, :]
    nc.vector.tensor_scalar(
        t, t, mv8[:, b, 0:1], rstd8[:, b:b + 1],
        op0=OP.subtract, op1=OP.mult)
    if use_gamma:
        nc.vector.tensor_mul(t, t, gamma_b[:])
    if use_beta:
        nc.vector.tensor_add(t, t, beta_b[:])


def _r(ap):
    """View a DRAM fp32 AP as float32r for DMA into f32r tiles."""
    return ap.bitcast(f32r)


def _build(flags, repeat=1):
    use_b1, use_b2, use_gamma, use_beta, use_bout = flags
    nc = _Bacc("TRN2", target_bir_lowering=False, debug=False,
               num_devices=NCORES)

    tokens = nc.declare_dram_parameter("tokens", [L], i32, isOutput=False)
    embed = nc.declare_dram_parameter("embed", [V, F], f32, isOutput=False)
    Wq = nc.declare_dram_parameter("Wq", [NL, F, HQ], f32, isOutput=False)
    Wk = nc.declare_dram_parameter("Wk", [NL, F, H * KD], f32, isOutput=False)
    Wv = nc.declare_dram_parameter("Wv", [NL, F, H * KD], f32, isOutput=False)
    W1 = nc.declare_dram_parameter("W1", [NL, HQ, F], f32, isOutput=False)
    b1 = nc.declare_dram_parameter("b1", [NL, F], f32, isOutput=False)
    W2 = nc.declare_dram_parameter("W2", [NL, F, F], f32, isOutput=False)
    b2 = nc.declare_dram_parameter("b2", [NL, F], f32, isOutput=False)
    gamma = nc.declare_dram_parameter("gamma", [NL, F], f32, isOutput=False)
    beta = nc.declare_dram_parameter("beta", [NL, F], f32, isOutput=False)
    Wout = nc.declare_dram_parameter("Wout", [F, V], f32, isOutput=False)
    bout = nc.declare_dram_parameter("bout", [V], f32, isOutput=False)
    out = nc.declare_dram_parameter("out", [L, V], f32, isOutput=True)

    with tile.TileContext(nc) as tc:
        with (
            tc.tile_pool(name="bigT", bufs=3) as bigT,    # [P, FC, L] f32r
            tc.tile_pool(name="nat", bufs=3) as natp,     # [P, LB, F] f32
            tc.tile_pool(name="qp", bufs=1) as qp,        # [P, LB, H, 65] f32
            tc.tile_pool(name="expp", bufs=EXPP_BUFS) as expp,  # [P, 512] f16
            tc.tile_pool(name="wp", bufs=4) as wp,        # [P, FC, 512] f32r
            tc.tile_pool(name="cst", bufs=1) as cst,
            tc.tile_pool(name="sm", bufs=16) as sm,       # small per-partition scalars
            tc.tile_pool(name="op", bufs=4) as outp,      # [P, 512] out staging
            tc.tile_pool(name="pp", bufs=PSUM_CFG[0], space="PSUM") as pp,
            tc.tile_pool(name="pa", bufs=PSUM_CFG[1], space="PSUM") as pa,
            tc.tile_pool(name="pt", bufs=PSUM_CFG[2], space="PSUM") as pt,
        ):
            # ---- constants ----
            ident = cst.tile([P, P], f32, tag="ident")
            make_identity(nc, ident[:])
            tri = cst.tile([P, P], f16, tag="tri")  # keep j<=i (upper incl diag)
            make_upper_triangular(nc, tri[:], val=1.0, diag=True)
            eps_t = cst.tile([P, 1], f32, tag="eps")
            nc.vector.memset(eps_t[:], 1e-5)
            neg5_t = cst.tile([P, 1], f32, tag="neg5")
            nc.vector.memset(neg5_t[:], -5.0)
            if use_b1:
                b1_sb = cst.tile([P, NL, FC], f32, tag="b1")
                nc.sync.dma_start(b1_sb[:], b1.rearrange("l (c p) -> p l c", p=P))
            if use_bout:
                bout_b = cst.tile([P, V], f32, tag="bout")
                bout_ap = bout[:]
                nc.sync.dma_start(
                    bout_b[:],
                    bass.AP(tensor=bout_ap.tensor, offset=bout_ap.offset,
                            ap=[[0, P]] + bout_ap.ap),
                )

            def bcast_row(dram_row_ap, tag):
                t = cst.tile([P, F], f32, tag=tag)
                nc.sync.dma_start(
                    t[:],
                    bass.AP(tensor=dram_row_ap.tensor, offset=dram_row_ap.offset,
                            ap=[[0, P]] + dram_row_ap.ap),
                )
                return t

            import contextlib
            _loop = (tc.For_i(0, repeat, 1) if repeat > 1
                     else contextlib.nullcontext())
            with _loop:
                # ---- embedding gather ----
                tok_sb = cst.tile([P, LB], i32, tag="tok")
                nc.sync.dma_start(tok_sb[:], tokens.rearrange("(b p) -> p b", p=P))
                x_nat = natp.tile([P, LB, F], f32, tag="nat")
                for b in range(LB):
                    nc.gpsimd.indirect_dma_start(
                        out=x_nat[:, b, :], out_offset=None,
                        in_=embed[:],
                        in_offset=bass.IndirectOffsetOnAxis(ap=tok_sb[:, b:b + 1], axis=0),
                    )

                def transpose_to_T(src_nat, dst_T):
                    """src natural [P, LB, F] f32 -> dst T [P, FC, L] f32r."""
                    if ABLATE == "transposes":
                        nc.gpsimd.memset(dst_T[:], 0.1)
                        return
                    for b in range(LB):
                        for c in range(FC):
                            pt_ps = pt.tile([P, P], f32, tag="pt")
                            nc.tensor.transpose(
                                pt_ps[:], src_nat[:, b, c * P:(c + 1) * P], ident[:])
                            if TR_SPLIT and (b * FC + c) % 2 == 1:
                                nc.scalar.copy(
                                    dst_T[:, c, b * P:(b + 1) * P], pt_ps[:])
                            else:
                                nc.vector.tensor_copy(
                                    dst_T[:, c, b * P:(b + 1) * P], pt_ps[:])

                xT = bigT.tile([P, FC, L], f32r, tag="bigT")
                transpose_to_T(x_nat, xT)

                # ---- layers ----
                for li in range(NL):
                    wq_t = wp.tile([P, FC, HQ], f32r, tag="w")
                    wk_t = wp.tile([P, FC, HQ], f32r, tag="w")
                    wv_t = wp.tile([P, FC, HQ], f32r, tag="w")
                    nc.sync.dma_start(wq_t[:], _r(Wq[li].rearrange("(c p) o -> p c o", p=P)))
                    nc.sync.dma_start(wk_t[:], _r(Wk[li].rearrange("(c p) o -> p c o", p=P)))
                    nc.sync.dma_start(wv_t[:], _r(Wv[li].rearrange("(c p) o -> p c o", p=P)))
                    w1_t = wp.tile([P, FC, F], f32r, tag="w")
                    w2_t = wp.tile([P, FC, F], f32r, tag="w")
                    nc.sync.dma_start(w1_t[:], _r(W1[li].rearrange("(c p) o -> p c o", p=P)))
                    nc.sync.dma_start(w2_t[:], _r(W2[li].rearrange("(c p) o -> p c o", p=P)))

                    # kT, vT projections (T layout out)
                    kT = bigT.tile([P, FC, L], f32r, tag="bigT")
                    vT = bigT.tile([P, FC, L], f32r, tag="bigT")
                    for w_t, oT in ((wk_t, kT), (wv_t, vT)):
                        for oc in range(FC):
                            for lc in range(2):
                                ps = pp.tile([P, 512], f32, tag="pp")
                                for fc in range(FC):
                                    nc.tensor.matmul(
                                        ps[:],
                                        w_t[:, fc, oc * P:(oc + 1) * P],
                                        xT[:, fc, lc * 512:(lc + 1) * 512],
                                        start=(fc == 0), stop=(fc == FC - 1))
                                nc.vector.tensor_copy(
                                    oT[:, oc, lc * 512:(lc + 1) * 512], ps[:])

                    # q natural (fp16 for the attend matmul), [P(j), jc, head, 65]
                    # with a trailing ones column so attend also yields row-sums
                    q_sb = qp.tile([P, LB, H, 65], f16, tag="q")
                    nc.vector.memset(q_sb[:, :, :, 64:65], 1.0)
                    for b in range(LB):
                        ps = pp.tile([P, 512], f32, tag="pp")
                        for fc in range(FC):
                            nc.tensor.matmul(
                                ps[:], xT[:, fc, b * P:(b + 1) * P], wq_t[:, fc, :],
                                start=(fc == 0), stop=(fc == FC - 1))
                        nc.scalar.copy(
                            q_sb[:, b, :, 0:64],
                            ps[:].rearrange("p (h d) -> p h d", h=H))

                    # attention, processed in head pairs so the K=64 score matmuls
                    # land on disjoint PE row-groups (partition bases 0 and 64) and
                    # run concurrently
                    x_new = natp.tile([P, LB, F], f32, tag="nat")
                    exp_store: dict = {}

                    def emit_scores(hpair, c):
                        heads = (2 * hpair, 2 * hpair + 1)
                        hc = hpair
                        tiles = {h: {} for h in heads}
                        for jc in range(4 * c + 4):
                            d = jc - 4 * c
                            n0 = 0 if d < 0 else min(P * d, 256)
                            e0 = 0 if d < 0 else P * d
                            pss = {}
                            for h in heads:
                                hb = 64 * (h % 2)
                                ps = pp.tile([P, 512], f32, tag="pp")
                                if ABLATE != "scores":
                                    nc.tensor.matmul(
                                        ps[:, n0:512],
                                        vT[hb:hb + KD, hc, jc * P:(jc + 1) * P],
                                        kT[hb:hb + KD, hc, c * 512 + n0:(c + 1) * 512],
                                        start=True, stop=True)
                                pss[h] = ps
                            for h in heads:
                                et = expp.tile([P, 512], f16, tag="exp")
                                # bias=-5: softmax is shift-invariant (both the
                                # attend numerator and the ones-column row-sum
                                # scale by e^-5), keeps exp within fp16 range
                                if ABLATE != "scores":
                                    nc.scalar.activation(
                                        et[:, e0:512], pss[h][:, e0:512], AF.Exp,
                                        bias=neg5_t[:])
                                    if d >= 0:
                                        # zero att where j > i (in-place
                                        # triangle select; 1-input gpsimd)
                                        nc.gpsimd.affine_select(
                                            out=et[:, e0:e0 + P],
                                            in_=et[:, e0:e0 + P],
                                            compare_op=OP.is_ge,
                                            fill=0.0, base=0,
                                            pattern=[[1, P]],
                                            channel_multiplier=-1)
                                else:
                                    nc.gpsimd.memset(et[:, e0:512], 0.5)
                                tiles[h][jc] = et
                        exp_store[(hpair, c)] = tiles

                    def emit_attend(hpair, c):
                        heads = (2 * hpair, 2 * hpair + 1)
                        tiles = exp_store.pop((hpair, c))
                        for b in range(4 * c, 4 * c + 4):
                            lc0 = (b - 4 * c) * P
                            if ABLATE == "attend":
                                for h in heads:
                                    nc.gpsimd.memset(
                                        x_new[:, b, h * 64:(h + 1) * 64], 0.1)
                                continue
                            # both heads of the pair accumulate into one
                            # psum bank: head h' at cols [65*h', 65*h'+65)
                            pa_ps = pa.tile([P, 130], f32, tag="pa")
                            for hi, h in enumerate(heads):
                                for jc in range(b + 1):
                                    nc.tensor.matmul(
                                        pa_ps[:, 65 * hi:65 * hi + 65],
                                        tiles[h][jc][:, lc0:lc0 + P],
                                        q_sb[:, jc, h, :],
                                        start=(jc == 0), stop=(jc == b))
                            pa2 = pa_ps[:].rearrange("p (h x) -> p h x", h=2)
                            rc = sm.tile([P, 2], f32, tag="rc")
                            nc.vector.reciprocal(rc[:], pa2[:, :, 64])
                            # x_new[:, b, heads] = att_u @ q * recip (recip
                            # broadcast 64-wide per head via 0-stride read)
                            xdst = x_new[:, b, :].rearrange(
                                "p (h x) -> p h x", h=H)[:, heads[0]:heads[0] + 2, :]
                            nc.vector.tensor_tensor(
                                xdst, pa2[:, :, 0:64],
                                rc[:, :, None].to_broadcast((P, 2, 64)),
                                OP.mult)

                    for p in range(H // 2):
                        emit_scores(p, 0)
                        emit_scores(p, 1)
                        emit_attend(p, 0)
                        emit_attend(p, 1)

                    # x_newT
                    x_newT = bigT.tile([P, FC, L], f32r, tag="bigT")
                    transpose_to_T(x_new, x_newT)

                    # MLP1: h1T = relu(W1^T x_newT + b1)
                    h1T = bigT.tile([P, FC, L], f32r, tag="bigT")
                    for oc in range(FC):
                        bias = b1_sb[:, li, oc:oc + 1] if use_b1 else 0.0
                        for lc in range(2):
                            ps = pp.tile([P, 512], f32, tag="pp")
                            for fc in range(FC):
                                nc.tensor.matmul(
                                    ps[:],
                                    w1_t[:, fc, oc * P:(oc + 1) * P],
                                    x_newT[:, fc, lc * 512:(lc + 1) * 512],
                                    start=(fc == 0), stop=(fc == FC - 1))
                            nc.scalar.activation(
                                h1T[:, oc, lc * 512:(lc + 1) * 512], ps[:],
                                AF.Relu, bias=bias)

                    # MLP2 + residual + LN -> y
                    if use_b2:
                        b2_b = bcast_row(b2[li], f"b2_{li}")
                    if use_gamma:
                        gamma_b = bcast_row(gamma[li], f"g_{li}")
                    if use_beta:
                        beta_b = bcast_row(beta[li], f"be_{li}")
                    y = natp.tile([P, LB, F], f32, tag="nat")
                    mv8 = sm.tile([P, LB, 2], f32, tag="mv8")
                    rstd8 = sm.tile([P, LB], f32, tag="rs8")
                    for b in range(LB):
                        ps = pp.tile([P, 512], f32, tag="pp")
                        for fc in range(FC):
                            nc.tensor.matmul(
                                ps[:],
                                h1T[:, fc, b * P:(b + 1) * P],
                                w2_t[:, fc, :],
                                start=(fc == 0), stop=(fc == FC - 1))
                        t = y[:, b, :]
                        nc.vector.tensor_add(t, ps[:], x_nat[:, b, :])
                        if use_b2:
                            nc.vector.tensor_add(t, t, b2_b[:])
                        st = sm.tile([P, 6], f32, tag="st")
                        nc.vector.bn_stats(st[:], t)
                        nc.vector.bn_aggr(mv8[:, b, :], st[:])
                        if not LN_BATCH:
                            # rstd = exp(-0.5*ln(var+eps)) per chunk: the layer
                            # tail stays pipelined (normalize+transpose of
                            # chunk b doesn't wait for later chunks' stats)
                            nc.scalar.activation(
                                rstd8[:, b:b + 1], mv8[:, b, 1:2], AF.Ln,
                                bias=eps_t[:])
                            nc.scalar.activation(
                                rstd8[:, b:b + 1], rstd8[:, b:b + 1], AF.Exp,
                                scale=-0.5)
                            _ln_apply(nc, y, b, mv8, rstd8, use_gamma, use_beta,
                                      gamma_b if use_gamma else None,
                                      beta_b if use_beta else None)
                    if LN_BATCH:
                        nc.scalar.activation(rstd8[:], mv8[:, :, 1], AF.Ln,
                                             bias=eps_t[:])
                        nc.scalar.activation(rstd8[:], rstd8[:], AF.Exp,
                                             scale=-0.5)
                        for b in range(LB):
                            _ln_apply(nc, y, b, mv8, rstd8, use_gamma, use_beta,
                                      gamma_b if use_gamma else None,
                                      beta_b if use_beta else None)

                    x_nat = y
                    xT = bigT.tile([P, FC, L], f32r, tag="bigT")
                    transpose_to_T(x_nat, xT)

                # ---- unembed ----
                wo = []
                for vc in range(2):
                    wt = wp.tile([P, FC, 512], f32r, tag="w")
                    nc.sync.dma_start(
                        wt[:],
                        _r(Wout[:, vc * 512:(vc + 1) * 512]
                           .rearrange("(c p) o -> p c o", p=P)))
                    wo.append(wt)
                for b in range(LB):
                    for vc in range(2):
                        ps = pp.tile([P, 512], f32, tag="pp")
                        for fc in range(FC):
                            nc.tensor.matmul(
                                ps[:],
                                xT[:, fc, b * P:(b + 1) * P],
                                wo[vc][:, fc, :],
                                start=(fc == 0), stop=(fc == FC - 1))
                        ot = outp.tile([P, 512], f32, tag="o")
                        if use_bout:
                            nc.vector.tensor_add(
                                ot[:], ps[:], bout_b[:, vc * 512:(vc + 1) * 512])
                        else:
                            nc.scalar.copy(ot[:], ps[:])
                        nc.sync.dma_start(
                            out[b * P:(b + 1) * P, vc * 512:(vc + 1) * 512], ot[:])
    nc.compile()
    return nc


def _get_nc(flags, repeat=1):
    key = (flags, repeat, ABLATE, LN_BATCH, PSUM_CFG, TR_SPLIT, EXPP_BUFS)
    if key not in _NC_CACHE:
        _NC_CACHE[key] = _build(flags, repeat)
    return _NC_CACHE[key]


def make_runner(flags, in_maps, repeat=1):
    """Build a reusable jitted SPMD runner with device-resident inputs.

    Returns (run, split_outputs) where run() executes the kernel once on all
    8 cores and blocks; used by test.py for timing without per-call host->device
    input transfer.
    """
    import jax
    from jax.sharding import Mesh, PartitionSpec, NamedSharding
    from concourse import bass2jax, mybir as _mybir

    bass2jax.install_neuronx_cc_hook()
    nc = _get_nc(flags, repeat)
    partition_name = (nc.partition_id_tensor.name if nc.partition_id_tensor
                      else None)
    in_names, out_names, out_avals, zero_outs = [], [], [], []
    for alloc in nc.m.functions[0].allocations:
        if not isinstance(alloc, _mybir.MemoryLocationSet):
            continue
        name = alloc.memorylocations[0].name
        if alloc.kind == "ExternalInput":
            if name != partition_name:
                in_names.append(name)
        elif alloc.kind == "ExternalOutput":
            shape = tuple(alloc.tensor_shape)
            dtype = _mybir.dt.np(alloc.dtype)
            out_names.append(name)
            out_avals.append(jax.core.ShapedArray(shape, dtype))
            zero_outs.append(np.zeros(shape, dtype))
    n_params = len(in_names)
    n_outs = len(out_avals)
    all_names = in_names + out_names + ([partition_name] if partition_name else [])

    def _body(*args):
        operands = list(args)
        if partition_name is not None:
            operands.append(bass2jax.partition_id_tensor())
        outs = bass2jax._bass_exec_p.bind(
            *operands,
            out_avals=tuple(out_avals),
            in_names=tuple(all_names),
            out_names=tuple(out_names),
            lowering_input_output_aliases=(),
            sim_require_finite=True,
            sim_require_nnan=True,
            nc=nc,
        )
        return tuple(outs)

    from jax.experimental.shard_map import shard_map
    devices = jax.devices()[:NCORES]
    mesh = Mesh(np.asarray(devices), ("core",))
    in_specs = (PartitionSpec("core"),) * (n_params + n_outs)
    out_specs = (PartitionSpec("core"),) * n_outs
    sharded = jax.jit(
        shard_map(_body, mesh=mesh, in_specs=in_specs, out_specs=out_specs,
                  check_rep=False),
        keep_unused=True,
    )
    concat_in = [
        np.concatenate([np.asarray(in_maps[c][nm])[None] for c in range(NCORES)],
                       axis=0).reshape(NCORES * np.asarray(in_maps[0][nm]).shape[0],
                                       *np.asarray(in_maps[0][nm]).shape[1:])
        for nm in in_names
    ]
    sh = NamedSharding(mesh, PartitionSpec("core"))
    dev_in = [jax.device_put(x, sh) for x in concat_in]
    dev_zeros = [
        jax.device_put(np.zeros((NCORES * z.shape[0], *z.shape[1:]), z.dtype), sh)
        for z in zero_outs
    ]

    def run():
        outs = sharded(*dev_in, *dev_zeros)
        jax.block_until_ready(outs)
        return outs

    def split(outs):
        return [
            {nm: np.asarray(outs[i]).reshape(NCORES, *out_avals[i].shape)[c]
             for i, nm in enumerate(out_names)}
            for c in range(NCORES)
        ]

    return run, split


def kernel(**inputs) -> np.ndarray:
    tokens = np.asarray(inputs["tokens"])
    args = {k: np.ascontiguousarray(np.asarray(v), dtype=np.float32)
            for k, v in inputs.items() if k != "tokens"}
    flags = (
        bool(np.any(args["b1"])),
        bool(np.any(args["b2"])),
        bool(np.any(args["gamma"] != 1.0)),
        bool(np.any(args["beta"])),
        bool(np.any(args["bout"])),
    )
    nc = _get_nc(flags)
    tok32 = np.ascontiguousarray(tokens.astype(np.int32))
    in_maps = [dict(args, tokens=tok32[c]) for c in range(NCORES)]
    res = run_bass_kernel_spmd(nc, in_maps, list(range(NCORES)))
    return np.stack([res.results[c]["out"] for c in range(NCORES)], axis=0)


if __name__ == "__main__":
    rng = np.random.default_rng(0)
    toy = {
        "tokens": rng.integers(0, V, size=(N, L)),
        "embed": rng.standard_normal((V, F)).astype(np.float32) * 0.02,
        "Wq": rng.standard_normal((NL, F, HQ)).astype(np.float32) * 0.02,
        "Wk": rng.standard_normal((NL, F, H * KD)).astype(np.float32) * 0.02,
        "Wv": rng.standard_normal((NL, F, H * KD)).astype(np.float32) * 0.02,
        "W1": rng.standard_normal((NL, HQ, F)).astype(np.float32) * 0.02,
        "b1": np.zeros((NL, F), np.float32),
        "W2": rng.standard_normal((NL, F, F)).astype(np.float32) * 0.02,
        "b2": np.zeros((NL, F), np.float32),
        "gamma": np.ones((NL, F), np.float32),
        "beta": np.zeros((NL, F), np.float32),
        "Wout": rng.standard_normal((F, V)).astype(np.float32) * 0.02,
        "bout": np.zeros((V,), np.float32),
    }
    o = kernel(**toy)
    print("out:", o.shape, o.dtype, float(np.abs(o).max()))

